# revision 7
# baseline (speedup 1.0000x reference)
"""Fused multi-head self-attention for Trainium2, SPMD over 8 NeuronCores. v2.

Problem (hardcoded): x [B=8, H=8, N=2048, C=64] f32, W_qkv [3C=192, C=64] f32.
    qkv = x @ W^T ; q,k,v = split(qkv, 3)
    attn = softmax(q @ k^T / sqrt(C), axis=-1) ; out = attn @ v
    head-mix: out.reshape(B,H,N,H,C//H).transpose(0,3,2,1,4).reshape(B,H,N,C)

Sharding: batch b -> core b (head-mix only mixes heads within a batch).

Measured on HW (slope-timed): the kernel is PE-bound, not ACT-bound (exp
FD=1024 costs only ~740ns, ACT runs ~45% idle). PE matmul cost is dominated
by streams (N=512 f16 ~213ns) plus LDWEIGHTS exposure (~107ns for 128-col
weights, not hidden for same-row-group back-to-back matmuls).

Changes vs the v1 baseline (427us -> ~336us):
- kT/qT kept partition-duplicated ([128, N], rows 64-127 = rows 0-63),
  produced for free by widened projection weights [W|W]: one 64-row-mode
  matmul emits both halves, same DVE copy cost. This enables PE row tiling:
  score chunk k = tile (0,0) on q-slice 0 + tile (64,0) on q-slice 1,
  alternating row groups so weight loads overlap the other group's stream.
- x transposes run as 64-row-mode T0/T8 pairs (concurrent halves).
- All projection matmuls 64-row mode; AV + epilogue transposes full
  128-mode; deferred work is queued in mode-aware lanes (row vs full) and
  drained at matching points of the attention stream; backlog capped (>3)
  so head boundaries don't burst into PE bubbles.
- Last head streams each finalized 128-row context tile to DRAM
  immediately (one DMA per tile covering all 8 output heads), overlapping
  the output DMA with the remaining epilogue instead of a 2MB tail.

Rejected (measured): fp8 DoubleRow AV (rel err 2.2e-2 > 2e-2 gate), f16
epilogue accumulator drain (denominator overflows f16 -> inf on HW),
chunk-level T0/T8 alternation (2x in a pure-matmul micro but regresses
in-kernel), AV lag (neutral), FD=2048 exp and any 3rd score buffer (PSUM:
pool slots are bank-granular; 2x2 score + 2 accum + 2 scratch = all 8).
"""

import numpy as np
from contextlib import ExitStack

import concourse.bass as bass
import concourse.tile as tile
from concourse import bacc, mybir
from concourse.bass_utils import run_bass_kernel_spmd
from concourse.masks import make_identity

F32 = mybir.dt.float32
F16 = mybir.dt.float16

B = 8
H = 8
N = 2048
C = 64
NCORES = 8

_prog_cache = {}


def build_attention_program(heads=H, n_ctx=N, c_dim=C, loop_reps=None):
    nc = bacc.Bacc("TRN2", target_bir_lowering=False, debug=False,
                   num_devices=NCORES)

    x = nc.dram_tensor("x", [heads, n_ctx, c_dim], F32, kind="ExternalInput").ap()
    w = nc.dram_tensor("w", [3 * c_dim, c_dim], F32, kind="ExternalInput").ap()
    out = nc.dram_tensor("out", [heads, n_ctx, c_dim], F32, kind="ExternalOutput").ap()

    with tile.TileContext(nc) as tc:
        _build_tile_kernel(tc, x, w, out, heads, n_ctx, c_dim, loop_reps=loop_reps)

    nc.compile()
    return nc


def _build_tile_kernel(tc, x, w, out, heads, n_ctx, c_dim, loop_reps=None):
    nc = tc.nc
    NT = n_ctx // 128            # k-chunks of 128
    QB = min(1024, n_ctx)        # q-block width (PSUM-budget bound)
    NQB = n_ctx // QB
    QS = 512                     # matmul slice width
    NS = QB // QS
    CG = c_dim // heads          # head-mix group size
    scale = float(c_dim) ** -0.5
    C1 = c_dim + 1               # AV carries a ones column -> denominator row

    ctx = ExitStack()
    const = ctx.enter_context(tc.tile_pool(name="const", bufs=1))
    xpool = ctx.enter_context(tc.tile_pool(name="xin", bufs=2))
    tpool = ctx.enter_context(tc.tile_pool(name="tmats", bufs=2))
    vpool = ctx.enter_context(tc.tile_pool(name="vnat", bufs=2))
    ppool = ctx.enter_context(tc.tile_pool(name="probs", bufs=5))
    opool = ctx.enter_context(tc.tile_pool(name="osb", bufs=4))
    rpool = ctx.enter_context(tc.tile_pool(name="recip", bufs=4))
    apool = ctx.enter_context(tc.tile_pool(name="assembly", bufs=1))
    ps_sc = ctx.enter_context(tc.tile_pool(name="ps_sc", bufs=2, space="PSUM"))
    ps_ot = ctx.enter_context(tc.tile_pool(name="ps_ot", bufs=2, space="PSUM"))
    ps_scr = ctx.enter_context(tc.tile_pool(name="ps_scr", bufs=2, space="PSUM"))

    # --- one-time setup -------------------------------------------------
    warm = const.tile([128, 1], F32, tag="warm")
    nc.vector.memset(warm[:], 0.0)
    nc.scalar.activation(out=warm[:], in_=warm[:],
                         func=mybir.ActivationFunctionType.Exp)

    ident = const.tile([128, 128], F32, tag="ident")
    make_identity(nc, ident[:])
    ident16 = const.tile([128, 128], F16, tag="ident16")
    nc.vector.tensor_copy(ident16[:], ident[:])

    # W [3C, C] -> wt [C, 3C] f16 via full-mode PE transposes (setup only)
    w1 = const.tile([128, c_dim], F32, tag="w1")
    w2 = const.tile([3 * c_dim - 128, c_dim], F32, tag="w2")
    nc.sync.dma_start(out=w1[:], in_=w[0:128, :])
    nc.sync.dma_start(out=w2[:], in_=w[128:3 * c_dim, :])
    wt = const.tile([c_dim, 3 * c_dim], F16, tag="wt")
    wt_ps1 = ps_scr.tile([c_dim, 128], F32, tag="scr")
    nc.tensor.transpose(wt_ps1[:], w1[:], ident[:])
    nc.vector.tensor_copy(wt[:, 0:128], wt_ps1[:])
    n2 = 3 * c_dim - 128
    wt_ps2 = ps_scr.tile([c_dim, n2], F32, tag="scr")
    nc.tensor.transpose(wt_ps2[:], w2[:], ident[0:n2, 0:n2])
    nc.vector.tensor_copy(wt[:, 128:3 * c_dim], wt_ps2[:])

    # widened projection weights: [Wq.T | Wq.T] and [Wk.T | Wk.T] so one
    # 64-row-mode matmul emits the projection partition-duplicated
    wq2 = const.tile([c_dim, 128], F16, tag="wq2")
    nc.vector.tensor_copy(wq2[:, 0:c_dim], wt[:, 0:c_dim])
    nc.vector.tensor_copy(wq2[:, c_dim:128], wt[:, 0:c_dim])
    wk2 = const.tile([c_dim, 128], F16, tag="wk2")
    nc.vector.tensor_copy(wk2[:, 0:c_dim], wt[:, c_dim:2 * c_dim])
    nc.vector.tensor_copy(wk2[:, c_dim:128], wt[:, c_dim:2 * c_dim])

    # persistent output assembly buffer [128, NT, H, C]
    asm = apool.tile([128, NT, heads, c_dim], F32, tag="asm")

    ones32 = const.tile([128, 1], F32, tag="ones32")
    nc.vector.memset(ones32[:], 1.0)
    ones_b = ones32[:]
    ones_bcast = bass.AP(tensor=ones_b.tensor, offset=ones_b.offset,
                         ap=[ones_b.ap[0], [0, NT], ones_b.ap[1]])

    def emit_body():
        # ---- projection for head h: mode-tagged thunk list ----
        # each thunk returns via closure; tagged 'row' (64-row-mode PE work),
        # 'any' (DVE/DMA only)
        def make_projection(h):
            xsb = xpool.tile([128, NT, c_dim], F32, tag="xsb", name=f"xsb_{h}")
            xsb16 = xpool.tile([128, NT, c_dim], F16, tag="xsb16",
                               name=f"xsb16_{h}")
            xT = tpool.tile([c_dim, n_ctx], F16, tag="xT", name=f"xT_{h}")
            kT2 = tpool.tile([128, n_ctx], F16, tag="kT2", name=f"kT2_{h}")
            qT2 = tpool.tile([128, n_ctx], F16, tag="qT2", name=f"qT2_{h}")
            vsb = vpool.tile([128, NT, C1], F16, tag="vsb", name=f"vsb_{h}")
            thunks = []  # (mode, fn)

            def dma_in():
                xr = x[h].rearrange("(q t p) c -> q p t c", q=4, p=128)
                for q in range(4):
                    sl = slice(q * (NT // 4), (q + 1) * (NT // 4))
                    nc.sync.dma_start(out=xsb[:, sl, :], in_=xr[q])
                    nc.vector.tensor_copy(xsb16[:, sl, :], xsb[:, sl, :])
                nc.vector.tensor_copy(vsb[:, :, c_dim:C1], ones_bcast)
            # goes on the 'row' queue so it drains BEFORE the xt/proj/vn
            # thunks that read xsb16/xT (queues preserve order within a key)
            thunks.append(("row", dma_in))

            # xT via 64-row-mode transpose pairs: chunk t rows 0-63 -> T0,
            # rows 64-127 -> T8 (both outputs land at PSUM partitions 0-63
            # in different scratch tiles/banks)
            xTr = xT[:].rearrange("c (t half n) -> c t half n", half=2, n=64)

            def xt_piece(g):
                pA = ps_scr.tile([c_dim, 4, 64], F16, tag="scr",
                                 name=f"xtA_{h}_{g}")
                pB = ps_scr.tile([c_dim, 4, 64], F16, tag="scr",
                                 name=f"xtB_{h}_{g}")
                for j in range(4):
                    t = g * 4 + j
                    nc.tensor.transpose(pA[:, j, :], xsb16[0:64, t, :],
                                        ident16[0:64, 0:64],
                                        tile_position=(0, 0))
                    nc.tensor.transpose(pB[:, j, :], xsb16[64:128, t, :],
                                        ident16[64:128, 64:128],
                                        tile_position=(64, 0))
                nc.vector.tensor_copy(xTr[:, g * 4:(g + 1) * 4, 0, :], pA[:])
                nc.vector.tensor_copy(xTr[:, g * 4:(g + 1) * 4, 1, :], pB[:])
            for g in range(NT // 4):
                thunks.append(("row", lambda g=g: xt_piece(g)))

            # kT2/qT2: one 64-row-mode matmul per 512-slice with widened
            # weights -> [128, 512] partition-duplicated, one DVE copy out
            def proj_piece(dst, w2t, s):
                pr = ps_scr.tile([128, QS], F32, tag="scr",
                                 name=f"pr_{h}_{id(dst) & 0xffff}_{s}")
                nc.tensor.matmul(pr[:], w2t[:], xT[:, s * QS:(s + 1) * QS],
                                 start=True, stop=True, tile_position=(0, 0))
                nc.vector.tensor_copy(dst[:, s * QS:(s + 1) * QS], pr[:])
            for s in range(n_ctx // QS):
                thunks.append(("row", lambda s=s: proj_piece(kT2, wk2, s)))
            for s in range(n_ctx // QS):
                thunks.append(("row", lambda s=s: proj_piece(qT2, wq2, s)))

            # v natural chunks via 64-row mode: lhsT = xT chunk [64,128]
            def vn_batch(g):
                vn_ps = ps_scr.tile([128, 4 * c_dim], F32, tag="scr",
                                    name=f"vn_{h}_{g}")
                for j in range(4):
                    t = g * 4 + j
                    nc.tensor.matmul(vn_ps[:, j * c_dim:(j + 1) * c_dim],
                                     xT[:, t * 128:(t + 1) * 128],
                                     wt[:, 2 * c_dim:3 * c_dim],
                                     start=True, stop=True,
                                     tile_position=(0, 0))
                nc.vector.tensor_copy(
                    vsb[:, g * 4:(g + 1) * 4, 0:c_dim],
                    vn_ps[:].rearrange("p (t c) -> p t c", c=c_dim))
            for g in range(NT // 4):
                thunks.append(("row", lambda g=g: vn_batch(g)))

            return thunks, (kT2, qT2, vsb)

        # ---- deferred-work queues (mode-aware) ----
        pending_row = []   # (key, fn) 64-row-mode PE work (next head's proj)
        pending_full = []  # (key, fn) full-mode PE / DVE work (epilogue, dma)

        def pop_one(q):
            key, fn = q.pop(0)
            fn()

        def drain_key(h):
            for q in (pending_row, pending_full):
                while any(k == ("proj", h) for k, _ in q):
                    pop_one(q)

        def emit_head(h, proj, on_qb_done=None):
            kT2, qT2, vsb = proj
            drain_key(h)  # this head's projection fully emitted

            def emit_drain(qb, ots):
                oTs = []
                for s in range(NS):
                    oT = opool.tile([C1, QS], F32, tag="oT",
                                    name=f"oT_{h}_{qb}_{s}")
                    nc.vector.tensor_copy(oT[:], ots[s][:])
                    oTs.append(oT)
                return oTs

            def norm_step(qb, oTs, s, j):
                q0 = qb * QB
                t = (q0 + s * QS) // 128 + j
                on_ps = ps_scr.tile([128, C1], F32, tag="scr",
                                    name=f"on_{h}_{qb}_{s}_{j}")
                nc.tensor.transpose(on_ps[:], oTs[s][:, j * 128:(j + 1) * 128],
                                    ident[0:C1, 0:C1])
                rec = rpool.tile([128, 1], F32, tag="rec",
                                 name=f"rec_{h}_{qb}_{s}_{j}")
                nc.vector.reciprocal(rec[:], on_ps[:, c_dim:C1])
                nc.vector.tensor_scalar_mul(
                    asm[:, t, :, h * CG:(h + 1) * CG],
                    on_ps[:, 0:c_dim].rearrange("p (a g) -> p a g", g=CG),
                    rec[:],
                )
                if h == heads - 1:
                    # last head: this 128-row context tile is now final for
                    # every output head -> stream it out immediately so the
                    # output DMA overlaps the remaining epilogue
                    dst = out.rearrange("h2 (t p) c -> p t h2 c", p=128)
                    nc.sync.dma_start(out=dst[:, t:t + 1, :, :],
                                      in_=asm[:, t:t + 1, :, :])

            def enqueue_epilogue(qb, oTs):
                for s in range(NS):
                    for j in range(QS // 128):
                        pending_full.append((("epi", h, qb),
                                             lambda s=s, j=j: norm_step(qb, oTs, s, j)))
                if on_qb_done is not None:
                    pending_full.append((("dma", h, qb),
                                         lambda: on_qb_done(qb)))

            for qb in range(NQB):
                q0 = qb * QB
                ots = [ps_ot.tile([C1, QS], F32, tag="ot",
                                  name=f"ot_h{h}_qb{qb}_s{s}")
                       for s in range(NS)]
                pts = [None] * NT

                def emit_scores(k):
                    # row-tiled: T0 computes q-slice 0, T8 q-slice 1 of the
                    # same chunk, concurrently into the 2 banks of sc_ps
                    sc_ps = ps_sc.tile([128, QB], F32, tag="sc",
                                       name=f"sc_{h}_{qb}_{k}")
                    nc.tensor.matmul(
                        sc_ps[:, 0:QS],
                        kT2[0:64, k * 128:(k + 1) * 128],
                        qT2[0:64, q0:q0 + QS],
                        start=True, stop=True, tile_position=(0, 0))
                    nc.tensor.matmul(
                        sc_ps[:, QS:QB],
                        kT2[64:128, k * 128:(k + 1) * 128],
                        qT2[64:128, q0 + QS:q0 + QB],
                        start=True, stop=True, tile_position=(64, 0))
                    pt = ppool.tile([128, QB], F16, tag="pt",
                                    name=f"pt_{h}_{qb}_{k}")
                    nc.scalar.activation(out=pt[:], in_=sc_ps[:],
                                         func=mybir.ActivationFunctionType.Exp,
                                         scale=scale)
                    pts[k] = pt

                def emit_av(k):
                    for s in range(NS):
                        nc.tensor.matmul(ots[s][:],
                                         vsb[:, k, :],
                                         pts[k][:, s * QS:(s + 1) * QS],
                                         start=(k == 0), stop=(k == NT - 1))

                # groups of 2 chunks: [row: sc(2i), row-deferred, sc(2i+1)]
                # then [full: av(2i-2), av(2i-1), full-deferred]
                for i in range(NT // 2):
                    emit_scores(2 * i)
                    if pending_row:
                        pop_one(pending_row)
                    emit_scores(2 * i + 1)
                    if i > 0:
                        emit_av(2 * i - 2)
                        emit_av(2 * i - 1)
                    if pending_full:
                        pop_one(pending_full)
                    elif pending_row:
                        pop_one(pending_row)
                    # keep the backlog from bursting at head boundaries
                    if len(pending_row) + len(pending_full) > 3:
                        pop_one(pending_row if len(pending_row) >= len(pending_full)
                                else pending_full)
                emit_av(NT - 2)
                emit_av(NT - 1)
                enqueue_epilogue(qb, emit_drain(qb, ots))

        # ---- pipeline over heads ----
        thunks, proj = make_projection(0)
        # critical chain for head 0: dma, xT, kT, qT emitted upfront
        n_crit = 1 + NT // 4 + 2 * (n_ctx // QS)
        for mode, t in thunks[:n_crit]:
            t()
        for mode, t in thunks[n_crit:]:
            (pending_row if mode == "row" else pending_full).append((("proj", 0), t))

        for h in range(heads):
            if h + 1 < heads:
                next_thunks, next_proj = make_projection(h + 1)
                for mode, t in next_thunks:
                    q = pending_row if mode == "row" else pending_full
                    q.append((("proj", h + 1), t))
            else:
                next_proj = None
            emit_head(h, proj)
            proj = next_proj
        while pending_row:
            pop_one(pending_row)
        while pending_full:
            pop_one(pending_full)

    if loop_reps:
        with tc.For_i(0, loop_reps, 1):
            emit_body()
    else:
        emit_body()

    ctx.close()


def _get_program():
    key = (H, N, C)
    if key not in _prog_cache:
        _prog_cache[key] = build_attention_program(*key)
    return _prog_cache[key]


def kernel(x: np.ndarray, W_qkv: np.ndarray) -> np.ndarray:
    x = np.ascontiguousarray(np.asarray(x, dtype=np.float32))
    W_qkv = np.ascontiguousarray(np.asarray(W_qkv, dtype=np.float32))
    assert x.shape == (B, H, N, C), x.shape
    assert W_qkv.shape == (3 * C, C), W_qkv.shape

    nc = _get_program()
    in_maps = [{"x": x[b], "w": W_qkv} for b in range(B)]
    res = run_bass_kernel_spmd(nc, in_maps, core_ids=list(range(NCORES)))
    outs = [res.results[b]["out"] for b in range(B)]
    return np.stack(outs, axis=0)


if __name__ == "__main__":
    xs = np.random.randn(B, H, N, C).astype(np.float32)
    ws = (np.random.randn(3 * C, C) * C ** -0.5).astype(np.float32)
    y = kernel(x=xs, W_qkv=ws)
    print("kernel output", y.shape, y.dtype, float(np.abs(y).mean()))
